# revision 23
# baseline (speedup 1.0000x reference)
"""BitStackLinear Trainium2 kernel (v4).

Computes out = x @ w.T where w = sum_i sign_i * (u_i @ vt_i), signs unpacked
from 4 packed bit-planes (one byte = 8 signs, little-endian).

Strategy: tensor-parallel over out_features across 8 NeuronCores
(1376 rows each). Per core:

  Phase R (reconstruct w.T into SBUF as bf16, per 128-row k-slab):
    - PE: r_i = vt_i.T @ u_i.T (rank-16 bf16 matmuls, 4 bits row-tiled at
      tile_position (32i, 0)) -> PSUM [128k, 1376o] f32, double-buffered
      (pr bufs=2) so matmuls pipeline against the drains.  vt columns carry
      the 2^(1-k%8) compensation for the byte-sign trick.
    - bytes arrive pre-masked (host ANDs bit j=k%8 into the replicated
      plane, same byte-level prep as the np.repeat): a_i in {0, 2^j}.
    - sign apply t_i = (a_i - 2^(j-1)) * r_i, engine-balanced:
        bit 0/1: DVE STT reading r straight from PSUM (fused drain+sign).
        bit 2:   ScalarE drains r2 to bf16; GpSimd multiplies by the sign
                 tensor s2 built on ScalarE (Identity, bias=-2^(j-1)).
        bit 3:   ScalarE drains r3; DVE bf16 multiply by s3.
      combine: A01 on DVE, A23 on GpSimd, final on DVE.
    - w.T slab resident in SBUF (bf16, 88KB/partition total).
  Phase G (GEMM, bf16 operands, f32 PSUM accumulation):
    - out.T[o, m] chunk = sum_k w.T[k, o-tile]^T-contraction @ xb[k, m-chunk]
    - o-tiles processed in PAIRS with alternating PSUM banks so consecutive
      matmuls never target the same bank back-to-back.
    - x is pre-cast to bf16 on the host (bit-identical to an on-device
      ScalarE cast, halves x DMA) and DMA'd straight into SBUF.
    - overlap: KOUTER o-tiles of m-chunk 0 accumulate k-outer DURING Phase R.
"""

import contextlib
import numpy as np

import concourse.bass as bass
import concourse.bacc as bacc
import concourse.mybir as mybir
import concourse.tile as tile

W_BIT = 4
OUT_F = 11008
IN_F = 4096
RANK = 16
NCORES = 8
O_SHARD = OUT_F // NCORES          # 1376
O_TILES = (O_SHARD + 127) // 128   # 11 (last tile 96 wide)
K_TILES = IN_F // 128              # 32
MC = 512                           # m-chunk width
KOUTER = 2                         # mb0 o-tiles accumulated k-outer in recon


def _bitstack_body(tc, aps, M):
    nc = tc.nc
    xT, qbE, uTb, vtpb, hm, nhm, outT = (
        aps["xT"], aps["qbE"], aps["uTb"], aps["vtpb"], aps["hm"], aps["nhm"],
        aps["outT"],
    )
    f32, u8, i32 = mybir.dt.float32, mybir.dt.uint8, mybir.dt.int32
    bf16 = mybir.dt.bfloat16
    AF = mybir.ActivationFunctionType
    OP = mybir.AluOpType
    n_mb = M // MC
    OS = O_SHARD

    with contextlib.ExitStack() as ctx:
        pool = ctx.enter_context(tc.tile_pool(name="sb", bufs=1))
        psum = ctx.enter_context(tc.tile_pool(name="ps", bufs=1, space="PSUM"))

        # ---- constants ----
        hm_t = pool.tile([128, 1], f32, name="hm_t")
        nc.sync.dma_start(hm_t, hm)
        nhm_t = pool.tile([128, 1], f32, name="nhm_t")
        nc.sync.dma_start(nhm_t, nhm)

        # ---- one-time: vt / u arrive pre-packed bf16 (vt pre-scaled by the
        # exact power-of-two column compensation 2^(1-k%8)); single DMAs.
        vtb = pool.tile([128, IN_F], bf16, name="vtb")
        nc.sync.dma_start(vtb, vtpb)
        utb = pool.tile([128, OS], bf16, name="utb")
        nc.sync.dma_start(utb, uTb)

        # ---- x chunk loads: host-pre-cast bf16, DMA straight to SBUF ----
        xb = {}

        def emit_xload(mb, k):
            xbt = pool.tile([128, MC], bf16, name=f"xb{mb}_{k}", tag=f"xb{k}",
                            bufs=2)
            nc.sync.dma_start(xbt, xT[k * 128:(k + 1) * 128,
                                      mb * MC:(mb + 1) * MC])
            xb[(mb, k)] = xbt

        # PSUM plan: tag "prg" [128,1536] f32 bufs=2 (6 banks) for recon pr
        # AND half the Phase-G pg tiles (using cols :512); tag "pg" [128,512]
        # bufs=2 (2 banks) for the k-outer accumulators and the other half.
        def psum_pg(name, which):
            if which == 0:
                t = psum.tile([128, 1536], f32, name=name, tag="prg", bufs=2)
                return t[:, 0:MC]
            return psum.tile([128, MC], f32, name=name, tag="pg", bufs=2)

        pgko = [psum.tile([128, MC], f32, name=f"pgko{ot}", tag="pg",
                          bufs=2) for ot in range(KOUTER)]
        w_tiles = []

        def emit_kouter(kk):
            for ot in range(KOUTER):
                nc.tensor.matmul(
                    pgko[ot],
                    w_tiles[kk][:, ot * 128:(ot + 1) * 128],
                    xb[(0, kk)],
                    start=(kk == 0), stop=(kk == K_TILES - 1),
                )

        # ---- Phase R: reconstruct w.T slabs into SBUF (bf16) ----
        pend = {}
        for ks in range(K_TILES):
            bts = pool.tile([128, W_BIT * OS], u8, name=f"bts{ks}", tag="bts",
                            bufs=2)
            nc.sync.dma_start(bts, qbE[ks * 128:(ks + 1) * 128, :])
            # sign tensors for bits 1,2 on ScalarE: s = a - 2^(j-1)
            s12 = pool.tile([128, 2 * OS], bf16, name=f"s12_{ks}", tag="s12",
                            bufs=2)
            nc.scalar.activation(s12, bts[:, OS:3 * OS], AF.Identity,
                                 bias=nhm_t, scale=1.0)
            wsb = pool.tile([128, OS], bf16, name=f"w{ks}", tag=f"w{ks}",
                            bufs=1)
            rts = {}
            for i in range(W_BIT):
                prt = psum.tile([128, 1536], f32, name=f"pr{ks}_{i}",
                                tag="prg", bufs=2)
                pr = prt[:, 0:OS]
                for c0 in range(0, OS, 512):
                    c1 = min(c0 + 512, OS)
                    nc.tensor.matmul(
                        pr[:, c0:c1],
                        vtb[32 * i:32 * i + RANK, ks * 128:(ks + 1) * 128],
                        utb[32 * i:32 * i + RANK, c0:c1],
                        start=True, stop=True,
                        tile_position=(32 * i, 0),
                    )
                if i == 0 or i == 3:
                    # fused drain+sign on DVE, PSUM-direct
                    dst = wsb if i == 0 else pool.tile(
                        [128, OS], bf16, name=f"r{ks}_{i}", tag="rt", bufs=6)
                    nc.vector.scalar_tensor_tensor(
                        out=dst, in0=bts[:, i * OS:(i + 1) * OS], scalar=hm_t,
                        in1=pr, op0=OP.subtract, op1=OP.mult)
                    if i == 3:
                        rts[i] = dst
                else:
                    # plain drain on ScalarE; sign applied after via s12
                    r = pool.tile([128, OS], bf16, name=f"r{ks}_{i}",
                                  tag="rt", bufs=6)
                    nc.scalar.copy(r, pr)
                    rts[i] = r
                    if i == 1:
                        # bit 1: bf16 multiply + fold into w, both on DVE
                        nc.vector.tensor_tensor(out=r, in0=s12[:, 0:OS],
                                                in1=r, op=OP.mult)
                        nc.vector.tensor_tensor(out=wsb, in0=wsb, in1=r,
                                                op=OP.add)
                if i == 2 and ks >= 2:
                    emit_kouter(ks - 2)
            # bit 2 sign multiply + pair add on GpSimd
            nc.gpsimd.tensor_tensor(out=rts[2], in0=s12[:, OS:2 * OS],
                                    in1=rts[2], op=OP.mult)
            nc.gpsimd.tensor_tensor(out=rts[2], in0=rts[2], in1=rts[3],
                                    op=OP.add)
            # defer the final add (w += t2+t3) of the PREVIOUS slab: it waits
            # on the GpSimd chain, so giving it a slab of slack keeps this
            # slab's DVE drains off that cross-engine critical path.
            if ks >= 1:
                nc.vector.tensor_tensor(out=w_tiles[ks - 1],
                                        in0=w_tiles[ks - 1],
                                        in1=pend[ks - 1], op=OP.add)
            pend[ks] = rts[2]
            w_tiles.append(wsb)
            # interleave x chunk loads for mb0/mb1 with recon
            for j in (2 * ks, 2 * ks + 1):
                mb, k = divmod(j, K_TILES)
                if mb < n_mb:
                    emit_xload(mb, k)
        nc.vector.tensor_tensor(out=w_tiles[K_TILES - 1],
                                in0=w_tiles[K_TILES - 1],
                                in1=pend[K_TILES - 1], op=OP.add)

        # ---- transition: give the PE ready work while the last two slabs'
        # elementwise chains finish, then close the k-outer groups.
        def emit_gemm_mms(pg, mb, ot, ow, k0, k1):
            for k in range(k0, k1):
                nc.tensor.matmul(
                    pg[:ow],
                    w_tiles[k][:, ot * 128:ot * 128 + ow],
                    xb[(mb, k)],
                    start=(k == 0), stop=(k == K_TILES - 1),
                )

        def emit_evac(pg, ow, mb, ot):
            ob = pool.tile([128, MC], f32, name=f"ob{mb}_{ot}", tag="ob",
                           bufs=3)
            if ot % 2 == 0:
                nc.scalar.copy(ob[:ow], pg[:ow])
            else:
                nc.vector.tensor_copy(ob[:ow], pg[:ow])
            nc.sync.dma_start(
                outT[ot * 128:ot * 128 + ow, mb * MC:(mb + 1) * MC], ob[:ow])

        pg_t5 = psum_pg("pgt5", 0)
        emit_gemm_mms(pg_t5, 0, KOUTER, 128, 0, K_TILES - 4)
        emit_kouter(K_TILES - 2)
        emit_kouter(K_TILES - 1)
        emit_gemm_mms(pg_t5, 0, KOUTER, 128, K_TILES - 4, K_TILES)
        for ot in range(KOUTER):
            emit_evac(pgko[ot], 128, 0, ot)
        emit_evac(pg_t5, 128, 0, KOUTER)

        # ---- Phase G: paired o-tiles, alternating PSUM bank groups ----
        for mb in range(n_mb):
            pf = mb + 1
            if 2 <= pf < n_mb:
                for k in range(K_TILES):
                    emit_xload(pf, k)
            ots = list(range(KOUTER + 1 if mb == 0 else 0, O_TILES))
            pairs = [ots[i:i + 2] for i in range(0, len(ots), 2)]
            for pair in pairs:
                pgs = []
                for idx, ot in enumerate(pair):
                    ow = min(128, OS - ot * 128)
                    pgs.append((psum_pg(f"pg{mb}_{ot}", idx % 2), ot, ow))
                for k in range(K_TILES):
                    for pg, ot, ow in pgs:
                        nc.tensor.matmul(
                            pg[:ow],
                            w_tiles[k][:, ot * 128:ot * 128 + ow],
                            xb[(mb, k)],
                            start=(k == 0), stop=(k == K_TILES - 1),
                        )
                for pg, ot, ow in pgs:
                    emit_evac(pg, ow, mb, ot)
            for k in range(K_TILES):
                del xb[(mb, k)]


def build_bass(M=8192):
    nc = bacc.Bacc("TRN2", target_bir_lowering=False, debug=False)
    f32, u8 = mybir.dt.float32, mybir.dt.uint8
    bf16 = mybir.dt.bfloat16
    aps = {}
    aps["xT"] = nc.dram_tensor("xT", [IN_F, M], bf16,
                               kind="ExternalInput").ap()
    # sign bytes pre-masked and pre-replicated 8x along k on host:
    # qbE[k, i*1376 + c] = qweight byte for (bit i, out c, in k) & (1<<(k%8))
    aps["qbE"] = nc.dram_tensor("qbE", [IN_F, W_BIT * O_SHARD], u8,
                                kind="ExternalInput").ap()
    # u bit-planes packed at partitions 32i..32i+16 (zeros elsewhere), bf16
    aps["uTb"] = nc.dram_tensor("uTb", [128, O_SHARD], bf16,
                                kind="ExternalInput").ap()
    # vt bit-planes packed likewise, bf16, pre-scaled by 2^(1-k%8) columns
    aps["vtpb"] = nc.dram_tensor("vtpb", [128, IN_F], bf16,
                                 kind="ExternalInput").ap()
    aps["hm"] = nc.dram_tensor("hm", [128, 1], f32, kind="ExternalInput").ap()
    aps["nhm"] = nc.dram_tensor("nhm", [128, 1], f32,
                                kind="ExternalInput").ap()
    aps["outT"] = nc.dram_tensor("outT", [O_SHARD, M], f32,
                                 kind="ExternalOutput").ap()
    with tile.TileContext(nc) as tc:
        _bitstack_body(tc, aps, M)
    nc.compile()
    return nc


def prep_inputs(x, qweight, u, vt):
    """Host-side layout prep: transposes / dtype views / byte replication and
    masking / the bf16 pre-cast of x (bit-identical to an on-device cast)."""
    import ml_dtypes
    M = x.shape[0] * x.shape[1]
    xT = np.ascontiguousarray(
        x.reshape(M, IN_F).T.astype(ml_dtypes.bfloat16))
    qb = qweight.astype(np.uint8)  # values 0..255 stored in int32
    p = np.arange(128)
    hm = (2.0 ** ((p % 8) - 1.0)).astype(np.float32).reshape(128, 1)
    nhm = -hm
    # vt packed: partition 32i+r holds vt[i, r, :], columns pre-scaled by the
    # exact power of two 2^(1-k%8) (lossless in f32), then cast bf16
    csc = (2.0 ** (1.0 - (np.arange(IN_F) % 8))).astype(np.float32)
    vtpb = np.zeros((128, IN_F), ml_dtypes.bfloat16)
    for i in range(W_BIT):
        vtpb[32 * i:32 * i + RANK] = (vt[i] * csc).astype(ml_dtypes.bfloat16)
    mask8 = (np.uint8(1) << (np.arange(IN_F) % 8).astype(np.uint8))[:, None]
    in_maps = []
    qb_r = qb.reshape(W_BIT, OUT_F, IN_F // 8)
    for c in range(NCORES):
        sl = slice(c * O_SHARD, (c + 1) * O_SHARD)
        # [bit, 512 bytes, o] -> replicate each byte row 8x -> [4096, o]
        qbT = qb_r[:, sl, :].transpose(0, 2, 1)          # [4, 512, 1376]
        qbE = np.ascontiguousarray(
            np.repeat(qbT, 8, axis=1).transpose(1, 0, 2).reshape(
                IN_F, W_BIT * O_SHARD))
        qbE &= mask8
        uTb = np.zeros((128, O_SHARD), ml_dtypes.bfloat16)
        for i in range(W_BIT):
            uTb[32 * i:32 * i + RANK] = u[i, sl, :].T.astype(
                ml_dtypes.bfloat16)
        in_maps.append({
            "xT": xT, "qbE": qbE, "uTb": uTb, "vtpb": vtpb,
            "hm": hm, "nhm": nhm,
        })
    return in_maps


def _enable_ldw_opt():
    """No-op (kept for test.py compat)."""


def kernel(x, qweight, u, vt):
    from concourse import bass_utils
    _enable_ldw_opt()
    x = np.asarray(x)
    qweight = np.asarray(qweight)
    u = np.asarray(u)
    vt = np.asarray(vt)
    B, S, _ = x.shape
    M = B * S
    nc = build_bass(M)
    in_maps = prep_inputs(x, qweight, u, vt)
    res = bass_utils.run_bass_kernel_spmd(nc, in_maps, core_ids=list(range(NCORES)))
    out = np.empty((M, OUT_F), np.float32)
    for c in range(NCORES):
        out[:, c * O_SHARD:(c + 1) * O_SHARD] = res.results[c]["outT"].T
    return out.reshape(B, S, OUT_F)


if __name__ == "__main__":
    rng = np.random.default_rng(0)
    x = rng.standard_normal((4, 2048, IN_F)).astype(np.float32)
    qw = rng.integers(0, 256, size=(W_BIT, OUT_F * IN_F // 8)).astype(np.int32)
    uu = (rng.standard_normal((W_BIT, OUT_F, RANK)) * 0.05).astype(np.float32)
    vv = (rng.standard_normal((W_BIT, RANK, IN_F)) * 0.05).astype(np.float32)
    out = kernel(x=x, qweight=qw, u=uu, vt=vv)
    print(out.shape, out.dtype)


# revision 24
# speedup vs baseline: 1.0127x; 1.0127x over previous
"""BitStackLinear Trainium2 kernel (v8).

Computes out = x @ w.T where w = sum_i sign_i * (u_i @ vt_i), signs unpacked
from 4 packed bit-planes (one byte = 8 signs, little-endian).

Strategy: tensor-parallel over out_features across 8 NeuronCores
(1376 rows each). Per core, a TWO-PASS k-split GEMM overlaps the whole
weight reconstruction with real GEMM work:

  Phase R (reconstruct w.T into SBUF as bf16, per 128-row k-slab):
    - PE: r_i = vt_i.T @ u_i.T (rank-16 bf16 matmuls, 4 bits row-tiled at
      tile_position (32i, 0)) -> PSUM [128k, 1376o] f32, pr double-buffered.
      vt columns carry the exact 2^(1-k%8) compensation (pre-scaled bf16).
    - bytes arrive pre-masked (host ANDs bit j=k%8 into the replicated
      plane): a_i in {0, 2^j}.
    - sign apply t_i = (a_i - 2^(j-1)) * r_i, engine-balanced:
        bits 0,3: DVE STT reading r straight from PSUM (fused drain+sign).
        bits 1,2: ScalarE drains to bf16; signs built on ScalarE (Identity,
                  bias=-2^(j-1)); multiplies on DVE (bit1) / GpSimd (bit2).
      combine: A01 on DVE; A23 on GpSimd; the final add is DEFERRED one slab
      so the GpSimd chain never blocks the next slab's DVE drains.
  Phase G (GEMM, bf16 operands, f32 PSUM accumulation, two k-passes):
    - pass 1 (k-slabs 0..S1-1) runs for ALL (m,o) tiles INTERLEAVED with the
      reconstruction of slabs S1..31 - the PE stays busy with real GEMM work
      while the drain pipeline streams.  Partials go to a DRAM scratch.
    - pass 2 (k-slabs S1..31) accumulates in PSUM, then the evacuation adds
      the pass-1 partial back in on the DVE (PSUM + SBUF add, same cost as
      a plain drain) and streams out.T to DRAM.
    - o-tiles in pass 2 go in PAIRS with alternating PSUM bank groups.
    - x is pre-cast to bf16 on the host (bit-identical to an on-device
      cast, halves x DMA); each (m,k) x-chunk is DMA'd exactly once.
"""

import contextlib
import numpy as np

import concourse.bass as bass
import concourse.bacc as bacc
import concourse.mybir as mybir
import concourse.tile as tile

W_BIT = 4
OUT_F = 11008
IN_F = 4096
RANK = 16
NCORES = 8
O_SHARD = OUT_F // NCORES          # 1376
O_TILES = (O_SHARD + 127) // 128   # 11 (last tile 96 wide)
K_TILES = IN_F // 128              # 32
MC = 512                           # m-chunk width
S1 = 6                             # k-slabs in GEMM pass 1


def _bitstack_body(tc, aps, M):
    nc = tc.nc
    xT, qbE, uTb, vtpb, hm, nhm, outT = (
        aps["xT"], aps["qbE"], aps["uTb"], aps["vtpb"], aps["hm"], aps["nhm"],
        aps["outT"],
    )
    f32, u8 = mybir.dt.float32, mybir.dt.uint8
    bf16 = mybir.dt.bfloat16
    AF = mybir.ActivationFunctionType
    OP = mybir.AluOpType
    n_mb = M // MC
    OS = O_SHARD

    with contextlib.ExitStack() as ctx:
        pool = ctx.enter_context(tc.tile_pool(name="sb", bufs=1))
        psum = ctx.enter_context(tc.tile_pool(name="ps", bufs=1, space="PSUM"))
        dpool = ctx.enter_context(tc.tile_pool(name="dr", bufs=1,
                                               space="DRAM"))
        # DRAM scratch for the pass-1 partial of out.T
        ptl = dpool.tile([OS, M], f32, name="ptl")

        # ---- constants ----
        hm_t = pool.tile([128, 1], f32, name="hm_t")
        nc.sync.dma_start(hm_t, hm)
        nhm_t = pool.tile([128, 1], f32, name="nhm_t")
        nc.sync.dma_start(nhm_t, nhm)

        # ---- one-time: vt / u arrive pre-packed bf16; single DMAs ----
        vtb = pool.tile([128, IN_F], bf16, name="vtb")
        nc.sync.dma_start(vtb, vtpb)
        utb = pool.tile([128, OS], bf16, name="utb")
        nc.sync.dma_start(utb, uTb)

        # ---- x chunk loads: host-pre-cast bf16, DMA straight to SBUF ----
        xb = {}

        def emit_xload(mb, k):
            xbt = pool.tile([128, MC], bf16, name=f"xb{mb}_{k}", tag=f"xb{k}",
                            bufs=2)
            nc.sync.dma_start(xbt, xT[k * 128:(k + 1) * 128,
                                      mb * MC:(mb + 1) * MC])
            xb[(mb, k)] = xbt

        w_tiles = []

        # ---- pass-1 emission units (consumed between recon slabs) ----
        def p1_loads(mb):
            def f():
                for k in range(S1):
                    emit_xload(mb, k)
            return f

        def p1_group(mb, ot):
            def f():
                ow = min(128, OS - ot * 128)
                pg = psum.tile([128, MC], f32, name=f"q{mb}_{ot}", tag="pg",
                               bufs=2)
                for k in range(S1):
                    nc.tensor.matmul(
                        pg[:ow],
                        w_tiles[k][:, ot * 128:ot * 128 + ow],
                        xb[(mb, k)],
                        start=(k == 0), stop=(k == S1 - 1),
                    )
                ob = pool.tile([128, MC], f32, name=f"p1o{mb}_{ot}", tag="ob",
                               bufs=2)
                if ot % 2 == 0:
                    nc.scalar.copy(ob[:ow], pg[:ow])
                else:
                    nc.vector.tensor_copy(ob[:ow], pg[:ow])
                nc.sync.dma_start(
                    ptl[ot * 128:ot * 128 + ow, mb * MC:(mb + 1) * MC],
                    ob[:ow])
                if ot == O_TILES - 1:
                    for k in range(S1):
                        del xb[(mb, k)]
            return f

        p1_units = []
        for mb in range(n_mb):
            p1_units.append(p1_loads(mb))
            for ot in range(O_TILES):
                p1_units.append(p1_group(mb, ot))
        p1_pos = [0]

        def emit_p1(upto):
            upto = min(int(upto), len(p1_units))
            while p1_pos[0] < upto:
                p1_units[p1_pos[0]]()
                p1_pos[0] += 1

        # ---- Phase R: reconstruct w.T slabs; interleave pass-1 ----
        pend = {}
        for ks in range(K_TILES):
            bts = pool.tile([128, W_BIT * OS], u8, name=f"bts{ks}", tag="bts",
                            bufs=2)
            nc.sync.dma_start(bts, qbE[ks * 128:(ks + 1) * 128, :])
            # sign tensors for bits 1,2 on ScalarE: s = a - 2^(j-1)
            s12 = pool.tile([128, 2 * OS], bf16, name=f"s12_{ks}", tag="s12",
                            bufs=1)
            nc.scalar.activation(s12, bts[:, OS:3 * OS], AF.Identity,
                                 bias=nhm_t, scale=1.0)
            wsb = pool.tile([128, OS], bf16, name=f"w{ks}", tag=f"w{ks}",
                            bufs=1)
            rts = {}
            for i in range(W_BIT):
                prt = psum.tile([128, 1536], f32, name=f"pr{ks}_{i}",
                                tag="prg", bufs=2)
                pr = prt[:, 0:OS]
                for c0 in range(0, OS, 512):
                    c1 = min(c0 + 512, OS)
                    nc.tensor.matmul(
                        pr[:, c0:c1],
                        vtb[32 * i:32 * i + RANK, ks * 128:(ks + 1) * 128],
                        utb[32 * i:32 * i + RANK, c0:c1],
                        start=True, stop=True,
                        tile_position=(32 * i, 0),
                    )
                if i == 0 or i == 3:
                    # fused drain+sign on DVE, PSUM-direct
                    dst = wsb if i == 0 else pool.tile(
                        [128, OS], bf16, name=f"r{ks}_{i}", tag="rt", bufs=6)
                    nc.vector.scalar_tensor_tensor(
                        out=dst, in0=bts[:, i * OS:(i + 1) * OS], scalar=hm_t,
                        in1=pr, op0=OP.subtract, op1=OP.mult)
                    if i == 3:
                        rts[i] = dst
                else:
                    # plain drain on ScalarE; sign applied after via s12
                    r = pool.tile([128, OS], bf16, name=f"r{ks}_{i}",
                                  tag="rt", bufs=6)
                    nc.scalar.copy(r, pr)
                    rts[i] = r
                    if i == 1:
                        # bit 1: bf16 multiply + fold into w, both on DVE
                        nc.vector.tensor_tensor(out=r, in0=s12[:, 0:OS],
                                                in1=r, op=OP.mult)
                        nc.vector.tensor_tensor(out=wsb, in0=wsb, in1=r,
                                                op=OP.add)
            # bit 2 sign multiply + pair add on GpSimd
            nc.gpsimd.tensor_tensor(out=rts[2], in0=s12[:, OS:2 * OS],
                                    in1=rts[2], op=OP.mult)
            nc.gpsimd.tensor_tensor(out=rts[2], in0=rts[2], in1=rts[3],
                                    op=OP.add)
            # deferred final add of the PREVIOUS slab (see docstring)
            if ks >= 1:
                nc.vector.tensor_tensor(out=w_tiles[ks - 1],
                                        in0=w_tiles[ks - 1],
                                        in1=pend[ks - 1], op=OP.add)
            pend[ks] = rts[2]
            w_tiles.append(wsb)
            if ks >= S1:
                # mb0/mb1 pass-2 x chunks trickle in during recon
                emit_xload(0, ks)
                emit_xload(1, ks)
            if ks >= S1 + 1:
                emit_p1(len(p1_units) * (ks - S1) / (K_TILES - S1 - 1))
        nc.vector.tensor_tensor(out=w_tiles[K_TILES - 1],
                                in0=w_tiles[K_TILES - 1],
                                in1=pend[K_TILES - 1], op=OP.add)
        emit_p1(len(p1_units))

        # ---- Phase G pass 2: paired o-tiles, fused partial-add evac ----
        def psum_pg(name, which):
            if which == 0:
                t = psum.tile([128, 1536], f32, name=name, tag="prg", bufs=2)
                return t[:, 0:MC]
            return psum.tile([128, MC], f32, name=name, tag="pg", bufs=2)

        for mb in range(n_mb):
            pf = mb + 1
            if 2 <= pf < n_mb:
                for k in range(S1, K_TILES):
                    emit_xload(pf, k)
            ots = list(range(O_TILES))
            pairs = [ots[i:i + 2] for i in range(0, len(ots), 2)]
            for pair in pairs:
                pgs = []
                for idx, ot in enumerate(pair):
                    ow = min(128, OS - ot * 128)
                    ppt = pool.tile([128, MC], f32, name=f"pp{mb}_{ot}",
                                    tag="pp", bufs=4)
                    nc.sync.dma_start(
                        ppt[:ow],
                        ptl[ot * 128:ot * 128 + ow, mb * MC:(mb + 1) * MC])
                    pgs.append((psum_pg(f"pg{mb}_{ot}", idx % 2), ot, ow, ppt))
                for k in range(S1, K_TILES):
                    for pg, ot, ow, ppt in pgs:
                        nc.tensor.matmul(
                            pg[:ow],
                            w_tiles[k][:, ot * 128:ot * 128 + ow],
                            xb[(mb, k)],
                            start=(k == S1), stop=(k == K_TILES - 1),
                        )
                for pg, ot, ow, ppt in pgs:
                    ob = pool.tile([128, MC], f32, name=f"ob{mb}_{ot}",
                                   tag="ob", bufs=2)
                    nc.vector.tensor_tensor(out=ob[:ow], in0=pg[:ow],
                                            in1=ppt[:ow], op=OP.add)
                    nc.sync.dma_start(
                        outT[ot * 128:ot * 128 + ow, mb * MC:(mb + 1) * MC],
                        ob[:ow])
            for k in range(S1, K_TILES):
                del xb[(mb, k)]


def build_bass(M=8192):
    nc = bacc.Bacc("TRN2", target_bir_lowering=False, debug=False)
    f32, u8 = mybir.dt.float32, mybir.dt.uint8
    bf16 = mybir.dt.bfloat16
    aps = {}
    aps["xT"] = nc.dram_tensor("xT", [IN_F, M], bf16,
                               kind="ExternalInput").ap()
    # sign bytes pre-masked and pre-replicated 8x along k on host:
    # qbE[k, i*1376 + c] = qweight byte for (bit i, out c, in k) & (1<<(k%8))
    aps["qbE"] = nc.dram_tensor("qbE", [IN_F, W_BIT * O_SHARD], u8,
                                kind="ExternalInput").ap()
    # u bit-planes packed at partitions 32i..32i+16 (zeros elsewhere), bf16
    aps["uTb"] = nc.dram_tensor("uTb", [128, O_SHARD], bf16,
                                kind="ExternalInput").ap()
    # vt bit-planes packed likewise, bf16, pre-scaled by 2^(1-k%8) columns
    aps["vtpb"] = nc.dram_tensor("vtpb", [128, IN_F], bf16,
                                 kind="ExternalInput").ap()
    aps["hm"] = nc.dram_tensor("hm", [128, 1], f32, kind="ExternalInput").ap()
    aps["nhm"] = nc.dram_tensor("nhm", [128, 1], f32,
                                kind="ExternalInput").ap()
    aps["outT"] = nc.dram_tensor("outT", [O_SHARD, M], f32,
                                 kind="ExternalOutput").ap()
    with tile.TileContext(nc) as tc:
        _bitstack_body(tc, aps, M)
    nc.compile()
    return nc


def prep_inputs(x, qweight, u, vt):
    """Host-side layout prep: transposes / dtype views / byte replication and
    masking / the bf16 pre-cast of x (bit-identical to an on-device cast)."""
    import ml_dtypes
    M = x.shape[0] * x.shape[1]
    xT = np.ascontiguousarray(
        x.reshape(M, IN_F).T.astype(ml_dtypes.bfloat16))
    qb = qweight.astype(np.uint8)  # values 0..255 stored in int32
    p = np.arange(128)
    hm = (2.0 ** ((p % 8) - 1.0)).astype(np.float32).reshape(128, 1)
    nhm = -hm
    # vt packed: partition 32i+r holds vt[i, r, :], columns pre-scaled by the
    # exact power of two 2^(1-k%8) (lossless in f32), then cast bf16
    csc = (2.0 ** (1.0 - (np.arange(IN_F) % 8))).astype(np.float32)
    vtpb = np.zeros((128, IN_F), ml_dtypes.bfloat16)
    for i in range(W_BIT):
        vtpb[32 * i:32 * i + RANK] = (vt[i] * csc).astype(ml_dtypes.bfloat16)
    mask8 = (np.uint8(1) << (np.arange(IN_F) % 8).astype(np.uint8))[:, None]
    in_maps = []
    qb_r = qb.reshape(W_BIT, OUT_F, IN_F // 8)
    for c in range(NCORES):
        sl = slice(c * O_SHARD, (c + 1) * O_SHARD)
        # [bit, 512 bytes, o] -> replicate each byte row 8x -> [4096, o]
        qbT = qb_r[:, sl, :].transpose(0, 2, 1)          # [4, 512, 1376]
        qbE = np.ascontiguousarray(
            np.repeat(qbT, 8, axis=1).transpose(1, 0, 2).reshape(
                IN_F, W_BIT * O_SHARD))
        qbE &= mask8
        uTb = np.zeros((128, O_SHARD), ml_dtypes.bfloat16)
        for i in range(W_BIT):
            uTb[32 * i:32 * i + RANK] = u[i, sl, :].T.astype(
                ml_dtypes.bfloat16)
        in_maps.append({
            "xT": xT, "qbE": qbE, "uTb": uTb, "vtpb": vtpb,
            "hm": hm, "nhm": nhm,
        })
    return in_maps


def _enable_ldw_opt():
    """No-op (kept for test.py compat)."""


def kernel(x, qweight, u, vt):
    from concourse import bass_utils
    _enable_ldw_opt()
    x = np.asarray(x)
    qweight = np.asarray(qweight)
    u = np.asarray(u)
    vt = np.asarray(vt)
    B, S, _ = x.shape
    M = B * S
    nc = build_bass(M)
    in_maps = prep_inputs(x, qweight, u, vt)
    res = bass_utils.run_bass_kernel_spmd(nc, in_maps, core_ids=list(range(NCORES)))
    out = np.empty((M, OUT_F), np.float32)
    for c in range(NCORES):
        out[:, c * O_SHARD:(c + 1) * O_SHARD] = res.results[c]["outT"].T
    return out.reshape(B, S, OUT_F)


if __name__ == "__main__":
    rng = np.random.default_rng(0)
    x = rng.standard_normal((4, 2048, IN_F)).astype(np.float32)
    qw = rng.integers(0, 256, size=(W_BIT, OUT_F * IN_F // 8)).astype(np.int32)
    uu = (rng.standard_normal((W_BIT, OUT_F, RANK)) * 0.05).astype(np.float32)
    vv = (rng.standard_normal((W_BIT, RANK, IN_F)) * 0.05).astype(np.float32)
    out = kernel(x=x, qweight=qw, u=uu, vt=vv)
    print(out.shape, out.dtype)


# revision 29
# speedup vs baseline: 1.0530x; 1.0397x over previous
"""BitStackLinear Trainium2 kernel (v8).

Computes out = x @ w.T where w = sum_i sign_i * (u_i @ vt_i), signs unpacked
from 4 packed bit-planes (one byte = 8 signs, little-endian).

Strategy: tensor-parallel over out_features across 8 NeuronCores
(1376 rows each). Per core, a TWO-PASS k-split GEMM overlaps the whole
weight reconstruction with real GEMM work:

  Phase R (reconstruct w.T into SBUF as bf16, per 128-row k-slab):
    - PE: r_i = vt_i.T @ u_i.T (rank-16 bf16 matmuls, 4 bits row-tiled at
      tile_position (32i, 0)) -> PSUM [128k, 1376o] f32, pr double-buffered.
      vt columns carry the exact 2^(1-k%8) compensation (pre-scaled bf16).
    - bytes arrive pre-masked (host ANDs bit j=k%8 into the replicated
      plane): a_i in {0, 2^j}.
    - sign apply t_i = (a_i - 2^(j-1)) * r_i, engine-balanced:
        bits 0,3: DVE STT reading r straight from PSUM (fused drain+sign).
        bits 1,2: ScalarE drains to bf16; signs built on ScalarE (Identity,
                  bias=-2^(j-1)); multiplies on DVE (bit1) / GpSimd (bit2).
      combine: A01 on DVE; A23 on GpSimd; the final add is DEFERRED one slab
      so the GpSimd chain never blocks the next slab's DVE drains.
  Phase G (GEMM, bf16 operands, f32 PSUM accumulation, two k-passes):
    - pass 1 (k-slabs 0..S1-1) runs for ALL (m,o) tiles INTERLEAVED with the
      reconstruction of slabs S1..31 - the PE stays busy with real GEMM work
      while the drain pipeline streams.  Partials go to a DRAM scratch.
    - pass 2 (k-slabs S1..31) accumulates in PSUM, then the evacuation adds
      the pass-1 partial back in on the DVE (PSUM + SBUF add, same cost as
      a plain drain) and streams out.T to DRAM.
    - o-tiles in pass 2 go in PAIRS with alternating PSUM bank groups.
    - x is pre-cast to bf16 on the host (bit-identical to an on-device
      cast, halves x DMA); each (m,k) x-chunk is DMA'd exactly once.
"""

import contextlib
import numpy as np

import concourse.bass as bass
import concourse.bacc as bacc
import concourse.mybir as mybir
import concourse.tile as tile

W_BIT = 4
OUT_F = 11008
IN_F = 4096
RANK = 16
NCORES = 8
O_SHARD = OUT_F // NCORES          # 1376
O_TILES = (O_SHARD + 127) // 128   # 11 (last tile 96 wide)
K_TILES = IN_F // 128              # 32
MC = 512                           # m-chunk width
S1 = 6                             # k-slabs in GEMM pass 1


def _bitstack_body(tc, aps, M):
    nc = tc.nc
    xT, qbE, uTb, vtpb, hm, nhm, outT = (
        aps["xT"], aps["qbE"], aps["uTb"], aps["vtpb"], aps["hm"], aps["nhm"],
        aps["outT"],
    )
    f32, u8 = mybir.dt.float32, mybir.dt.uint8
    bf16 = mybir.dt.bfloat16
    AF = mybir.ActivationFunctionType
    OP = mybir.AluOpType
    n_mb = M // MC
    OS = O_SHARD

    with contextlib.ExitStack() as ctx:
        pool = ctx.enter_context(tc.tile_pool(name="sb", bufs=1))
        psum = ctx.enter_context(tc.tile_pool(name="ps", bufs=1, space="PSUM"))
        dpool = ctx.enter_context(tc.tile_pool(name="dr", bufs=1,
                                               space="DRAM"))
        # DRAM scratch for the pass-1 partial of out.T (bf16: the partial is
        # ~0.43|out|, so the 4e-3 bf16 rounding adds only ~1.7e-3 rel err)
        ptl = dpool.tile([OS, M], bf16, name="ptl")

        # ---- constants ----
        hm_t = pool.tile([128, 1], f32, name="hm_t")
        nc.sync.dma_start(hm_t, hm)
        nhm_t = pool.tile([128, 1], f32, name="nhm_t")
        nc.sync.dma_start(nhm_t, nhm)

        # ---- one-time: vt / u arrive pre-packed bf16; single DMAs ----
        vtb = pool.tile([128, IN_F], bf16, name="vtb")
        nc.sync.dma_start(vtb, vtpb)
        utb = pool.tile([128, OS], bf16, name="utb")
        nc.sync.dma_start(utb, uTb)

        # ---- x chunk loads: host-pre-cast bf16, DMA straight to SBUF ----
        xb = {}

        def emit_xload(mb, k):
            xbt = pool.tile([128, MC], bf16, name=f"xb{mb}_{k}", tag=f"xb{k}",
                            bufs=2)
            nc.sync.dma_start(xbt, xT[k * 128:(k + 1) * 128,
                                      mb * MC:(mb + 1) * MC])
            xb[(mb, k)] = xbt

        w_tiles = []

        # ---- pass-1 emission units (consumed between recon slabs) ----
        def p1_loads(mb):
            def f():
                for k in range(S1):
                    emit_xload(mb, k)
            return f

        def p1_group(mb, ot):
            def f():
                ow = min(128, OS - ot * 128)
                pg = psum.tile([128, MC], f32, name=f"q{mb}_{ot}", tag="pg",
                               bufs=2)
                for k in range(S1):
                    nc.tensor.matmul(
                        pg[:ow],
                        w_tiles[k][:, ot * 128:ot * 128 + ow],
                        xb[(mb, k)],
                        start=(k == 0), stop=(k == S1 - 1),
                    )
                ob = pool.tile([128, MC], bf16, name=f"p1o{mb}_{ot}",
                               tag="o1", bufs=3)
                if ot % 3 == 2:
                    nc.vector.tensor_copy(ob[:ow], pg[:ow])
                else:
                    nc.scalar.copy(ob[:ow], pg[:ow])
                nc.sync.dma_start(
                    ptl[ot * 128:ot * 128 + ow, mb * MC:(mb + 1) * MC],
                    ob[:ow])
                if ot == O_TILES - 1:
                    for k in range(S1):
                        del xb[(mb, k)]
            return f

        p1_units = []
        for mb in range(n_mb):
            p1_units.append(p1_loads(mb))
            for ot in range(O_TILES):
                p1_units.append(p1_group(mb, ot))
        p1_pos = [0]

        def emit_p1(upto):
            upto = min(int(upto), len(p1_units))
            while p1_pos[0] < upto:
                p1_units[p1_pos[0]]()
                p1_pos[0] += 1

        # ---- Phase R: reconstruct w.T slabs; interleave pass-1 ----
        pend = {}
        for ks in range(K_TILES):
            bts = pool.tile([128, W_BIT * OS], u8, name=f"bts{ks}", tag="bts",
                            bufs=2)
            nc.sync.dma_start(bts, qbE[ks * 128:(ks + 1) * 128, :])
            # sign tensors for bits 1,2 on ScalarE: s = a - 2^(j-1)
            s12 = pool.tile([128, 2 * OS], bf16, name=f"s12_{ks}", tag="s12",
                            bufs=1)
            nc.scalar.activation(s12, bts[:, OS:3 * OS], AF.Identity,
                                 bias=nhm_t, scale=1.0)
            wsb = pool.tile([128, OS], bf16, name=f"w{ks}", tag=f"w{ks}",
                            bufs=1)
            rts = {}
            for i in range(W_BIT):
                # feed the PE ~2 pass-1 units ahead of every recon matmul so
                # a pr-slot stall never leaves the in-order queue head empty
                if ks >= S1 + 1:
                    done = (ks - S1 - 1) * W_BIT + i
                    emit_p1(len(p1_units) * done
                            / ((K_TILES - S1 - 2) * W_BIT))
                prt = psum.tile([128, 1536], f32, name=f"pr{ks}_{i}",
                                tag="prg", bufs=2)
                pr = prt[:, 0:OS]
                for c0 in range(0, OS, 512):
                    c1 = min(c0 + 512, OS)
                    nc.tensor.matmul(
                        pr[:, c0:c1],
                        vtb[32 * i:32 * i + RANK, ks * 128:(ks + 1) * 128],
                        utb[32 * i:32 * i + RANK, c0:c1],
                        start=True, stop=True,
                        tile_position=(32 * i, 0),
                    )
                if i == 0 or i == 3:
                    # fused drain+sign on DVE, PSUM-direct
                    dst = wsb if i == 0 else pool.tile(
                        [128, OS], bf16, name=f"r{ks}_{i}", tag="rt", bufs=6)
                    nc.vector.scalar_tensor_tensor(
                        out=dst, in0=bts[:, i * OS:(i + 1) * OS], scalar=hm_t,
                        in1=pr, op0=OP.subtract, op1=OP.mult)
                    if i == 3:
                        rts[i] = dst
                else:
                    # plain drain on ScalarE; sign applied after via s12
                    r = pool.tile([128, OS], bf16, name=f"r{ks}_{i}",
                                  tag="rt", bufs=6)
                    nc.scalar.copy(r, pr)
                    rts[i] = r
                    if i == 1:
                        # bit 1: bf16 multiply + fold into w, both on DVE
                        nc.vector.tensor_tensor(out=r, in0=s12[:, 0:OS],
                                                in1=r, op=OP.mult)
                        nc.vector.tensor_tensor(out=wsb, in0=wsb, in1=r,
                                                op=OP.add)
            # bit 2 sign multiply + pair add on GpSimd
            nc.gpsimd.tensor_tensor(out=rts[2], in0=s12[:, OS:2 * OS],
                                    in1=rts[2], op=OP.mult)
            nc.gpsimd.tensor_tensor(out=rts[2], in0=rts[2], in1=rts[3],
                                    op=OP.add)
            # deferred final add of the PREVIOUS slab (see docstring)
            if ks >= 1:
                nc.vector.tensor_tensor(out=w_tiles[ks - 1],
                                        in0=w_tiles[ks - 1],
                                        in1=pend[ks - 1], op=OP.add)
            pend[ks] = rts[2]
            w_tiles.append(wsb)
            if ks >= S1:
                # mb0/mb1 pass-2 x chunks trickle in during recon
                emit_xload(0, ks)
                emit_xload(1, ks)
        nc.vector.tensor_tensor(out=w_tiles[K_TILES - 1],
                                in0=w_tiles[K_TILES - 1],
                                in1=pend[K_TILES - 1], op=OP.add)
        emit_p1(len(p1_units))

        # ---- Phase G pass 2: paired o-tiles, fused partial-add evac ----
        def psum_pg(name, which):
            if which == 0:
                t = psum.tile([128, 1536], f32, name=name, tag="prg", bufs=2)
                return t[:, 0:MC]
            return psum.tile([128, MC], f32, name=name, tag="pg", bufs=2)

        for mb in range(n_mb):
            pf = mb + 1
            if 2 <= pf < n_mb:
                for k in range(S1, K_TILES):
                    emit_xload(pf, k)
            ots = list(range(O_TILES))
            pairs = [ots[i:i + 2] for i in range(0, len(ots), 2)]
            for pair in pairs:
                pgs = []
                for idx, ot in enumerate(pair):
                    ow = min(128, OS - ot * 128)
                    ppt = pool.tile([128, MC], bf16, name=f"pp{mb}_{ot}",
                                    tag="pp", bufs=4)
                    nc.sync.dma_start(
                        ppt[:ow],
                        ptl[ot * 128:ot * 128 + ow, mb * MC:(mb + 1) * MC])
                    pgs.append((psum_pg(f"pg{mb}_{ot}", idx % 2), ot, ow, ppt))
                for k in range(S1, K_TILES):
                    for pg, ot, ow, ppt in pgs:
                        nc.tensor.matmul(
                            pg[:ow],
                            w_tiles[k][:, ot * 128:ot * 128 + ow],
                            xb[(mb, k)],
                            start=(k == S1), stop=(k == K_TILES - 1),
                        )
                for pg, ot, ow, ppt in pgs:
                    ob = pool.tile([128, MC], f32, name=f"ob{mb}_{ot}",
                                   tag="ob", bufs=2)
                    nc.vector.tensor_tensor(out=ob[:ow], in0=pg[:ow],
                                            in1=ppt[:ow], op=OP.add)
                    nc.sync.dma_start(
                        outT[ot * 128:ot * 128 + ow, mb * MC:(mb + 1) * MC],
                        ob[:ow])
            for k in range(S1, K_TILES):
                del xb[(mb, k)]


def build_bass(M=8192):
    nc = bacc.Bacc("TRN2", target_bir_lowering=False, debug=False)
    f32, u8 = mybir.dt.float32, mybir.dt.uint8
    bf16 = mybir.dt.bfloat16
    aps = {}
    aps["xT"] = nc.dram_tensor("xT", [IN_F, M], bf16,
                               kind="ExternalInput").ap()
    # sign bytes pre-masked and pre-replicated 8x along k on host:
    # qbE[k, i*1376 + c] = qweight byte for (bit i, out c, in k) & (1<<(k%8))
    aps["qbE"] = nc.dram_tensor("qbE", [IN_F, W_BIT * O_SHARD], u8,
                                kind="ExternalInput").ap()
    # u bit-planes packed at partitions 32i..32i+16 (zeros elsewhere), bf16
    aps["uTb"] = nc.dram_tensor("uTb", [128, O_SHARD], bf16,
                                kind="ExternalInput").ap()
    # vt bit-planes packed likewise, bf16, pre-scaled by 2^(1-k%8) columns
    aps["vtpb"] = nc.dram_tensor("vtpb", [128, IN_F], bf16,
                                 kind="ExternalInput").ap()
    aps["hm"] = nc.dram_tensor("hm", [128, 1], f32, kind="ExternalInput").ap()
    aps["nhm"] = nc.dram_tensor("nhm", [128, 1], f32,
                                kind="ExternalInput").ap()
    aps["outT"] = nc.dram_tensor("outT", [O_SHARD, M], f32,
                                 kind="ExternalOutput").ap()
    with tile.TileContext(nc) as tc:
        _bitstack_body(tc, aps, M)
    nc.compile()
    return nc


def prep_inputs(x, qweight, u, vt):
    """Host-side layout prep: transposes / dtype views / byte replication and
    masking / the bf16 pre-cast of x (bit-identical to an on-device cast)."""
    import ml_dtypes
    M = x.shape[0] * x.shape[1]
    xT = np.ascontiguousarray(
        x.reshape(M, IN_F).T.astype(ml_dtypes.bfloat16))
    qb = qweight.astype(np.uint8)  # values 0..255 stored in int32
    p = np.arange(128)
    hm = (2.0 ** ((p % 8) - 1.0)).astype(np.float32).reshape(128, 1)
    nhm = -hm
    # vt packed: partition 32i+r holds vt[i, r, :], columns pre-scaled by the
    # exact power of two 2^(1-k%8) (lossless in f32), then cast bf16
    csc = (2.0 ** (1.0 - (np.arange(IN_F) % 8))).astype(np.float32)
    vtpb = np.zeros((128, IN_F), ml_dtypes.bfloat16)
    for i in range(W_BIT):
        vtpb[32 * i:32 * i + RANK] = (vt[i] * csc).astype(ml_dtypes.bfloat16)
    mask8 = (np.uint8(1) << (np.arange(IN_F) % 8).astype(np.uint8))[:, None]
    in_maps = []
    qb_r = qb.reshape(W_BIT, OUT_F, IN_F // 8)
    for c in range(NCORES):
        sl = slice(c * O_SHARD, (c + 1) * O_SHARD)
        # [bit, 512 bytes, o] -> replicate each byte row 8x -> [4096, o]
        qbT = qb_r[:, sl, :].transpose(0, 2, 1)          # [4, 512, 1376]
        qbE = np.ascontiguousarray(
            np.repeat(qbT, 8, axis=1).transpose(1, 0, 2).reshape(
                IN_F, W_BIT * O_SHARD))
        qbE &= mask8
        uTb = np.zeros((128, O_SHARD), ml_dtypes.bfloat16)
        for i in range(W_BIT):
            uTb[32 * i:32 * i + RANK] = u[i, sl, :].T.astype(
                ml_dtypes.bfloat16)
        in_maps.append({
            "xT": xT, "qbE": qbE, "uTb": uTb, "vtpb": vtpb,
            "hm": hm, "nhm": nhm,
        })
    return in_maps


def _enable_ldw_opt():
    """No-op (kept for test.py compat)."""


def kernel(x, qweight, u, vt):
    from concourse import bass_utils
    _enable_ldw_opt()
    x = np.asarray(x)
    qweight = np.asarray(qweight)
    u = np.asarray(u)
    vt = np.asarray(vt)
    B, S, _ = x.shape
    M = B * S
    nc = build_bass(M)
    in_maps = prep_inputs(x, qweight, u, vt)
    res = bass_utils.run_bass_kernel_spmd(nc, in_maps, core_ids=list(range(NCORES)))
    out = np.empty((M, OUT_F), np.float32)
    for c in range(NCORES):
        out[:, c * O_SHARD:(c + 1) * O_SHARD] = res.results[c]["outT"].T
    return out.reshape(B, S, OUT_F)


if __name__ == "__main__":
    rng = np.random.default_rng(0)
    x = rng.standard_normal((4, 2048, IN_F)).astype(np.float32)
    qw = rng.integers(0, 256, size=(W_BIT, OUT_F * IN_F // 8)).astype(np.int32)
    uu = (rng.standard_normal((W_BIT, OUT_F, RANK)) * 0.05).astype(np.float32)
    vv = (rng.standard_normal((W_BIT, RANK, IN_F)) * 0.05).astype(np.float32)
    out = kernel(x=x, qweight=qw, u=uu, vt=vv)
    print(out.shape, out.dtype)
